# revision 6
# baseline (speedup 1.0000x reference)
"""LAN attention kernel for Trainium2, 8 NeuronCores, head-parallel.

Math (per head h, batch b; D=64, T=1024), all with per-row/per-col scalar
structure (i = query pos, j = key pos; layout: j on partitions, i on free):
    p = pq[i] + pk[j]   -> phi = sigmoid(p)
    w = wq[i] + wk[j]   -> t   = sigmoid(w)
    c = cq[i] + ck[j]   -> tau = softplus(c) = ln(1 + exp(c))   (eps dropped,
                           effect on logits < 1e-6)
    v = tau * t
    logits[j,i] = phi * t * (1 - exp(-v)) / v
    attn = softmax_j;  out = attn @ V;  y = sum_h out_h @ Wo_h + const

ACT passes per [T,T] tile: sigmoid x2 (sigmoid table set), Exp(c), Ln(e1+1),
Exp(-v), Exp(logits) (natural_log_exp set).  DVE: m=phi*t, v=sp*t,
r=recip_approx_fast(v), gneg=(e-1)*r, lneg=m*gneg (S = Exp(-lneg)).
Softmax denominator comes free from a ones-column in the S^T @ [V|1] matmul.

Host folds q/k projections into 6 per-head scalar vectors (exact algebra --
the same (Wphi_in@Wphi_out) folding the reference itself performs), sums the
8 partial outputs and adds the v/out bias constants.
"""

import numpy as np

B, T, DM, H, D = 4, 1024, 512, 8, 64
NCHUNK = T // 128          # 8 j-chunks per (b) tile
MCHUNK = (B * T) // 128    # 32 row chunks total

_CACHE = {}


def _f32(x):
    return np.ascontiguousarray(np.asarray(x, dtype=np.float32))


def _build_program():
    import concourse.bacc as bacc
    import concourse.mybir as mybir
    import concourse.tile as tile

    dt = mybir.dt
    AF = mybir.ActivationFunctionType
    ALU = mybir.AluOpType

    nc = bacc.Bacc("TRN2", target_bir_lowering=False, debug=False)

    xT_d = nc.dram_tensor("xT", [DM, B * T], dt.float32, kind="ExternalInput")
    wv_d = nc.dram_tensor("wv", [DM, D], dt.float32, kind="ExternalInput")
    wo_d = nc.dram_tensor("wo", [D, DM], dt.float32, kind="ExternalInput")
    # per-chunk per-partition biases: [32, 128, 3] = (pk', ck', wk')
    kb_d = nc.dram_tensor("kb", [MCHUNK, 128, 3], dt.float32, kind="ExternalInput")
    # q-side broadcast vectors: [B, 3, T] = (pq, cq, wq)
    qv_d = nc.dram_tensor("qv", [B, 3, T], dt.float32, kind="ExternalInput")
    out_d = nc.dram_tensor("out", [B, T, DM], dt.float32, kind="ExternalOutput")

    with tile.TileContext(nc) as tc:
        with (
            tc.tile_pool(name="const", bufs=1) as const,
            tc.tile_pool(name="xin", bufs=4) as xin,
            tc.tile_pool(name="vtile", bufs=1) as vtile,
            tc.tile_pool(name="bcast", bufs=1) as bcast,
            tc.tile_pool(name="persist", bufs=1) as persist,
            tc.tile_pool(name="work", bufs=2) as work,
            tc.tile_pool(name="norm", bufs=2) as norm,
            tc.tile_pool(name="outp", bufs=3) as outp,
            tc.tile_pool(name="ps_v", bufs=2, space="PSUM") as ps_v,
            tc.tile_pool(name="ps_o", bufs=1, space="PSUM") as ps_o,
            tc.tile_pool(name="ps_t", bufs=1, space="PSUM") as ps_t,
            tc.tile_pool(name="ps_w", bufs=2, space="PSUM") as ps_w,
        ):
            # ---- constants / small inputs ----
            wv_sb = const.tile([128, 4, D], dt.float32)
            nc.sync.dma_start(wv_sb[:], wv_d[:].rearrange("(c p) d -> p c d", p=128))
            wo_sb = const.tile([D, DM], dt.float32)
            nc.sync.dma_start(wo_sb[:], wo_d[:])
            kb_sb = const.tile([128, MCHUNK, 3], dt.float32)
            nc.sync.dma_start(kb_sb[:], kb_d[:].rearrange("c p v -> p c v"))
            one_sb = const.tile([1, 1], dt.float32)
            nc.vector.memset(one_sb[:], 1.0)

            # ---- V projection: v_sb[:, m, 0:64] = (x @ Wv_h) rows; col 64 = 1
            v_sb = vtile.tile([128, MCHUNK, D + 1], dt.float32)
            nc.vector.memset(v_sb[:], 1.0)
            for m in range(MCHUNK):
                xt_t = xin.tile([128, 4, 128], dt.float32, tag="xt")
                nc.sync.dma_start(
                    xt_t[:],
                    xT_d[:, m * 128 : (m + 1) * 128].rearrange(
                        "(c p) f -> p c f", p=128
                    ),
                )
                pv = ps_v.tile([128, D], dt.float32, tag="pv")
                for kc in range(4):
                    nc.tensor.matmul(
                        pv[:],
                        xt_t[:, kc, :],
                        wv_sb[:, kc, :],
                        start=(kc == 0),
                        stop=(kc == 3),
                    )
                nc.vector.tensor_copy(v_sb[:, m, 0:D], pv[:])

            # ---- attention per batch ----
            for b in range(B):
                pq_t = bcast.tile([128, T], dt.float32, tag="pq")
                cq_t = bcast.tile([128, T], dt.float32, tag="cq")
                wq_t = bcast.tile([128, T], dt.float32, tag="wq")
                nc.sync.dma_start(pq_t[:], qv_d[b, 0, :][None, :].to_broadcast((128, T)))
                nc.sync.dma_start(cq_t[:], qv_d[b, 1, :][None, :].to_broadcast((128, T)))
                nc.sync.dma_start(wq_t[:], qv_d[b, 2, :][None, :].to_broadcast((128, T)))

                # phase 1 (sigmoid table set): t = sigmoid(w), m = phi * t
                t_all = persist.tile([128, NCHUNK, T], dt.float32, tag="t_all")
                m_all = persist.tile([128, NCHUNK, T], dt.float32, tag="m_all")
                for jc in range(NCHUNK):
                    g = b * NCHUNK + jc
                    phi = work.tile([128, T], dt.float32, tag="phi")
                    nc.scalar.activation(
                        phi[:], pq_t[:], AF.Sigmoid, bias=kb_sb[:, g, 0:1], scale=1.0
                    )
                    nc.scalar.activation(
                        t_all[:, jc, :], wq_t[:], AF.Sigmoid,
                        bias=kb_sb[:, g, 2:3], scale=1.0,
                    )
                    nc.vector.tensor_tensor(
                        m_all[:, jc, :], phi[:], t_all[:, jc, :], op=ALU.mult
                    )

                # phase 2 (natural_log_exp table set)
                po = [
                    ps_o.tile([D + 1, 512], dt.float32, tag=f"po{ni}", name=f"po{ni}_{b}")
                    for ni in range(2)
                ]
                for jc in range(NCHUNK):
                    g = b * NCHUNK + jc
                    e1 = work.tile([128, T], dt.float32, tag="e1")
                    nc.scalar.activation(
                        e1[:], cq_t[:], AF.Exp, bias=kb_sb[:, g, 1:2], scale=1.0
                    )
                    sp = work.tile([128, T], dt.float32, tag="sp")
                    nc.scalar.activation(sp[:], e1[:], AF.Ln, bias=1.0, scale=1.0)
                    v_t = work.tile([128, T], dt.float32, tag="v_t")
                    nc.vector.tensor_tensor(
                        v_t[:], sp[:], t_all[:, jc, :], op=ALU.mult
                    )
                    e_t = work.tile([128, T], dt.float32, tag="e")
                    nc.scalar.activation(e_t[:], v_t[:], AF.Exp, scale=-1.0)
                    r1 = work.tile([128, T], dt.float32, tag="r1")
                    nc.vector.reciprocal_approx_fast(r1[:], v_t[:])
                    gn = work.tile([128, T], dt.float32, tag="gn")
                    nc.vector.scalar_tensor_tensor(
                        gn[:], e_t[:], 1.0, r1[:], op0=ALU.subtract, op1=ALU.mult
                    )
                    ln_t = work.tile([128, T], dt.float32, tag="ln_t")
                    nc.vector.tensor_tensor(
                        ln_t[:], m_all[:, jc, :], gn[:], op=ALU.mult
                    )
                    s_t = work.tile([128, T], dt.float32, tag="s")
                    nc.scalar.activation(s_t[:], ln_t[:], AF.Exp, scale=-1.0)
                    for ni in range(2):
                        nc.tensor.matmul(
                            po[ni][:],
                            v_sb[:, g, :],
                            s_t[:, ni * 512 : (ni + 1) * 512],
                            start=(jc == 0),
                            stop=(jc == NCHUNK - 1),
                        )

                # denominators -> per-i-chunk reciprocal column
                den_sb = norm.tile([1, T], dt.float32, tag="den")
                nc.vector.tensor_copy(den_sb[:, 0:512], po[0][D : D + 1, :])
                nc.vector.tensor_copy(den_sb[:, 512:T], po[1][D : D + 1, :])
                pdT = ps_t.tile([128, NCHUNK], dt.float32, tag="pdT")
                for ic in range(NCHUNK):
                    nc.tensor.matmul(
                        pdT[:, ic : ic + 1],
                        den_sb[:, ic * 128 : (ic + 1) * 128],
                        one_sb[:],
                        start=True,
                        stop=True,
                    )
                rdT = norm.tile([128, NCHUNK], dt.float32, tag="rdT")
                nc.vector.reciprocal_approx_fast(rdT[:], pdT[:])

                # unnormalized out^T -> SBUF (lhsT for the Wo matmul)
                oT = norm.tile([D, T], dt.float32, tag="oT")
                nc.vector.tensor_copy(oT[:, 0:512], po[0][0:D, :])
                nc.vector.tensor_copy(oT[:, 512:T], po[1][0:D, :])

                # partial = (out^T)^T @ Wo_h, normalized by rdT per row
                for ic in range(NCHUNK):
                    pw = ps_w.tile([128, DM], dt.float32, tag="pw")
                    nc.tensor.matmul(
                        pw[:],
                        oT[:, ic * 128 : (ic + 1) * 128],
                        wo_sb[:],
                        start=True,
                        stop=True,
                    )
                    ob = outp.tile([128, DM], dt.float32, tag="ob")
                    nc.vector.tensor_scalar(
                        ob[:], pw[:], rdT[:, ic : ic + 1], None, op0=ALU.mult
                    )
                    nc.sync.dma_start(
                        out_d[b, ic * 128 : (ic + 1) * 128, :], ob[:]
                    )

    nc.compile()
    return nc


def _get_program():
    if "nc" not in _CACHE:
        _CACHE["nc"] = _build_program()
    return _CACHE["nc"]


def _host_prep(inputs):
    x = _f32(inputs["x"]).reshape(B * T, DM)
    Wq, bq = _f32(inputs["Wq"]), _f32(inputs["bq"])
    Wk, bk = _f32(inputs["Wk"]), _f32(inputs["bk"])
    Wv, bv = _f32(inputs["Wv"]), _f32(inputs["bv"])
    Wo, bo = _f32(inputs["Wo"]), _f32(inputs["bo"])

    w_phi = (_f32(inputs["Wphi_in"]) @ _f32(inputs["Wphi_out"]))[:, 0]
    b_phi = float(_f32(inputs["bphi_in"]) @ _f32(inputs["Wphi_out"])[:, 0]
                  + _f32(inputs["bphi_out"])[0])
    w_tab = _f32(inputs["Wta"])[:, 0] + _f32(inputs["Wtb"])[:, 0]
    b_tab = float(_f32(inputs["bta"])[0] + _f32(inputs["btb"])[0])
    w_tau = (_f32(inputs["Wtau_in"]) @ _f32(inputs["Wtau_out"]))[:, 0]
    b_tau = float(_f32(inputs["btau_in"]) @ _f32(inputs["Wtau_out"])[:, 0]
                  + _f32(inputs["btau_out"])[0])

    xT = np.ascontiguousarray(x.T)  # [512, 4096]

    in_maps = []
    for h in range(H):
        hs = slice(h * D, (h + 1) * D)
        Wq_h, Wk_h = Wq[:, hs], Wk[:, hs]
        bq_h, bk_h = bq[hs], bk[hs]

        def pair_vecs(wvec, bconst):
            qv = x @ (Wq_h @ wvec[:D]) + float(bq_h @ wvec[:D])
            kv = x @ (Wk_h @ wvec[D:]) + float(bk_h @ wvec[D:]) + bconst
            return qv.astype(np.float32), kv.astype(np.float32)

        pq, pk = pair_vecs(w_phi, b_phi)
        cq, ck = pair_vecs(w_tau, b_tau)
        wq, wk = pair_vecs(w_tab, b_tab)

        kb = np.stack([pk, ck, wk], axis=-1)    # [4096, 3]
        qv_arr = np.stack([pq, cq, wq], axis=0)  # [3, 4096]

        in_maps.append({
            "xT": xT,
            "wv": np.ascontiguousarray(Wv[:, hs]),
            "wo": np.ascontiguousarray(Wo[hs, :]),
            "kb": np.ascontiguousarray(kb.reshape(MCHUNK, 128, 3)),
            "qv": np.ascontiguousarray(
                qv_arr.reshape(3, B, T).transpose(1, 0, 2)
            ),
        })

    extra = bv @ Wo + bo  # [512] constant fold of the v/out biases
    return in_maps, extra


def kernel(**inputs):
    from concourse.bass_utils import run_bass_kernel_spmd

    nc = _get_program()
    in_maps, extra = _host_prep(inputs)
    res = run_bass_kernel_spmd(nc, in_maps, list(range(H)))
    out = np.zeros((B, T, DM), dtype=np.float32)
    for r in res.results:
        out += np.asarray(r["out"], dtype=np.float32)
    out += extra[None, None, :]
    return out
